# revision 1
# baseline (speedup 1.0000x reference)
"""Trainium2 Bass kernel for weighted-CE + structural-penalty loss.

Full inputs -> data-parallel shard over batch across 8 NeuronCores ->
per-core Bass kernel computes small partial sums -> host combines the
(tiny) partials in float64.

CE:  -mean(w[t] * log_softmax(logits)[t]) = (1/N) sum_c w_c (W_c - S_c),
  W_c = sum_pos 1[t==c]*lse,  S_c = sum_pos 1[t==c]*x_c.
  An interleaved one-hot M[p, j*8+c] = (t==c) (fp16, one 2x-mode
  tensor_tensor per chunk from a GPSIMD-replicated int16 target) feeds:
   - lse side: matmul(lhsT=lse-block, rhs=M window) accumulating a
     shifted diagonal in one PSUM bank, classes separated by col%8;
   - x side:  MX = M * Xh elementwise, then ones-matmuls column-reduce
     into a [1, 512] PSUM (fold j%64, classes by col%8);
   - nnz: ones-matmul over M's class-0 stride-8 columns.
  Host extracts the diagonals/columns and applies weights in float64.

Penalty: per row, pen = pair_sum + P_final - 2*min(0, min_prefix(P)) with
  P = cumsum((s==1)-(s==2)) via the hardware tensor_tensor_scan; pair
  terms are shifted-mask products reduced by ones-matmuls.  Rows are
  split into two 2048-halves on partitions r | 64+r (first half has a
  3-column real halo, second a zero halo); host chains the halves and
  adds the one genuinely-clamped boundary term.
"""

import numpy as np

import concourse.bass as bass
import concourse.mybir as mybir
import concourse.tile as tile
from concourse import bacc
from concourse.bass_utils import run_bass_kernel_spmd

B, S, C = 512, 4096, 8
PENALTY_WEIGHT = 0.1
NCORES = 8
RB = B // NCORES          # rows (batch) per core
N = RB * S                # positions per core
P = 128                   # SBUF partitions
NP = N // P               # positions per partition
NCH = 4                   # CE processed in NCH free-dim chunks
PCH = NP // NCH           # positions per partition per chunk (512)
NW = PCH // 64            # 64-position rhs windows per chunk (8)
SH = S // 2               # penalty half-row length
HALO = 3

F32 = mybir.dt.float32
F16 = mybir.dt.float16
I32 = mybir.dt.int32
I16 = mybir.dt.int16
OP = mybir.AluOpType
AF = mybir.ActivationFunctionType


def _patch_act_tables():
    """Prefer the single table set containing Exp+Ln+Copy so the kernel
    pays one ACT_TABLE_LOAD instead of alternating per chunk.  Set ids
    are positional, so blank out other sets rather than reordering."""
    import concourse.hw_specs as hw_specs
    if getattr(hw_specs, "_loss_kernel_tables_patched", False):
        return
    orig = hw_specs.get_activation_tables

    def patched(arch):
        t = orig(arch)
        pref = "natural_log_exp_and_others"
        if pref not in t:
            return t
        return {k: (v if k == pref else set()) for k, v in t.items()}

    hw_specs.get_activation_tables = patched
    bacc.get_activation_tables = patched
    hw_specs._loss_kernel_tables_patched = True


USE_TABLE_PATCH = True


def build_program(compile=True):
    if USE_TABLE_PATCH:
        _patch_act_tables()
    nc = bacc.Bacc("TRN2", target_bir_lowering=False, debug=False)

    logits_d = nc.dram_tensor("logits", [P, NP * C], F32, kind="ExternalInput").ap()
    targets_d = nc.dram_tensor("targets", [P, NP], I32, kind="ExternalInput").ap()
    structs_d = nc.dram_tensor("structs", [RB, S], I32, kind="ExternalInput").ap()

    dlse_d = nc.dram_tensor("diag_lse", [64, 512], F32, kind="ExternalOutput").ap()
    dx_d = nc.dram_tensor("diag_x", [P, 8, P], F32, kind="ExternalOutput").ap()
    vec_d = nc.dram_tensor("vec_acc", [1, 4, 512], F32, kind="ExternalOutput").ap()
    pen_scan_d = nc.dram_tensor("pen_scan", [P, 2], F32, kind="ExternalOutput").ap()

    SW = SH + HALO

    with tile.TileContext(nc) as tc:
        with (
            tc.tile_pool(name="big", bufs=2) as big,
            tc.tile_pool(name="ebuf", bufs=1) as ebuf,
            tc.tile_pool(name="mid", bufs=1) as mid,
            tc.tile_pool(name="lsep", bufs=2) as lsep,
            tc.tile_pool(name="mip", bufs=2) as mip,
            tc.tile_pool(name="pen", bufs=1) as pen,
            tc.tile_pool(name="acc", bufs=1) as acc,
            tc.tile_pool(name="psum", bufs=1, space="PSUM") as psum,
        ):
            # psum accumulators
            ps_lse = psum.tile([64, 512], F32, name="ps_lse")
            ps_x = [psum.tile([P, 4, P], F32, name=f"ps_x{q}") for q in range(2)]
            ps_vec = [psum.tile([1, 512], F32, name=f"ps_vec{i}") for i in range(4)]
            # ps_vec: 0=cnt0, 1=pair2, 2=pair3, 3=pair4
            started = set()

            def acc_mm(key, out, lhsT, rhs, last):
                st = key not in started
                started.add(key)
                nc.tensor.matmul(out, lhsT=lhsT, rhs=rhs, start=st, stop=last)

            ones_t = acc.tile([P, 1], F16)
            nc.vector.memset(ones_t, 1.0)

            t_sb = pen.tile([P, NP], I32)
            nc.sync.dma_start(out=t_sb, in_=targets_d)

            # ---------------- CE chunks ----------------
            for k in range(NCH):
                fl = k * PCH * C
                x_t = big.tile([P, PCH * C], F32, tag="x")
                nc.sync.dma_start(out=x_t, in_=logits_d[:, fl : fl + PCH * C])

                # class-blocked masks first: DVE fills the exp wait
                m2 = mip.tile([P, C, PCH], F16, tag="m2")
                tk = t_sb[:, k * PCH : (k + 1) * PCH]
                for c in range(C):
                    nc.vector.tensor_scalar(out=m2[:, c, :], in0=tk,
                                            scalar1=float(c), scalar2=None,
                                            op0=OP.is_equal)

                e_x = ebuf.tile([P, PCH * C], F16, tag="e")
                nc.scalar.activation(e_x, x_t, AF.Exp)
                e3 = e_x.rearrange("p (n c) -> p n c", c=C)
                s4 = mid.tile([P, PCH, 4], F16, tag="s4")
                nc.vector.tensor_add(s4, e3[:, :, 0:4], e3[:, :, 4:8])
                s2 = mid.tile([P, PCH, 2], F16, tag="s2")
                nc.vector.tensor_add(s2, s4[:, :, 0:2], s4[:, :, 2:4])
                se = mid.tile([P, PCH], F16, tag="se")
                se3 = se.rearrange("p (n o) -> p n o", o=1)
                nc.vector.tensor_add(se3, s2[:, :, 0:1], s2[:, :, 1:2])
                lse = lsep.tile([P, PCH], F16, tag="lse")
                nc.scalar.activation(lse, se, AF.Ln)
                xh = ebuf.tile([P, PCH * C], F16, tag="xh")
                nc.scalar.activation(xh, x_t, AF.Copy)  # fp32 -> fp16 cast
                xh3 = xh.rearrange("p (n c) -> p n c", c=C)

                last = k == NCH - 1
                # lse side: 64-position windows; rhs gathers all 8 class
                # slices for the window -> permuted diagonal, all rows useful
                for w in range(NW):
                    rhs = bass.AP(
                        tensor=m2.tensor, offset=m2.offset + w * 64,
                        ap=[m2.ap[0], [PCH, C], [1, 64]])
                    acc_mm(("lse",), ps_lse,
                           lhsT=lse[:, w * 64 : (w + 1) * 64], rhs=rhs,
                           last=last and w == NW - 1)

                # x side: per-class diagonal psums (4 classes per bank)
                for c in range(C):
                    q, sl = divmod(c, 4)
                    for b in range(PCH // P):
                        bs = slice(b * P, (b + 1) * P)
                        acc_mm(("x", q), ps_x[q][:, sl, :],
                               lhsT=m2[:, c, bs], rhs=xh3[:, bs, c],
                               last=(last and c in (3, 7) and b == PCH // P - 1))

                # count of t==0: ones-matmul over the class-0 mask block
                acc_mm(("cnt",), ps_vec[0], lhsT=ones_t, rhs=m2[:, 0, :],
                       last=last)

            # -------- penalty: row halves on partitions (r | 64+r) --------
            s_t = pen.tile([P, SW], I32)
            nc.sync.dma_start(out=s_t[0:RB, :], in_=structs_d[:, 0:SW])
            nc.sync.dma_start(out=s_t[RB:P, 0:SH], in_=structs_d[:, SH:S])
            nc.vector.memset(s_t[RB:P, SH:SW], 0)

            lp_t = pen.tile([P, SW], F16)
            r_t = pen.tile([P, SW], F16)
            e_t = pen.tile([P, SW], F16)
            nc.vector.tensor_scalar(out=lp_t, in0=s_t, scalar1=1.0, scalar2=None,
                                    op0=OP.is_equal)
            nc.vector.tensor_scalar(out=r_t, in0=s_t, scalar1=2.0, scalar2=None,
                                    op0=OP.is_equal)
            nc.vector.tensor_scalar(out=e_t, in0=s_t, scalar1=3.0, scalar2=None,
                                    op0=OP.is_equal)

            p_t = pen.tile([P, SH], F32)
            nc.vector.tensor_tensor_scan(out=p_t, data0=lp_t[:, 0:SH],
                                         data1=r_t[:, 0:SH], initial=0.0,
                                         op0=OP.add, op1=OP.subtract)
            scan_out = acc.tile([P, 2], F32)
            nc.vector.tensor_copy(out=scan_out[:, 0:1], in_=p_t[:, SH - 1 : SH])
            nc.vector.tensor_reduce(out=scan_out[:, 1:2], in_=p_t,
                                    axis=mybir.AxisListType.X, op=OP.min)
            nc.sync.dma_start(out=pen_scan_d, in_=scan_out)

            # er[j]=e[j]*r[j+1]; eer[j]=e[j]*er[j+1]; pair products with lp
            er_t = pen.tile([P, SW], F16)
            eer_t = pen.tile([P, SW], F16)
            nc.vector.tensor_mul(er_t[:, 0 : SW - 1], e_t[:, 0 : SW - 1], r_t[:, 1:SW])
            nc.vector.tensor_mul(eer_t[:, 0 : SW - 2], e_t[:, 0 : SW - 2],
                                 er_t[:, 1 : SW - 1])
            pr2 = pen.tile([P, SH], F16)
            pr3 = pen.tile([P, SH], F16)
            pr4 = pen.tile([P, SH], F16)
            nc.vector.tensor_mul(pr2, lp_t[:, 0:SH], r_t[:, 1 : SH + 1])
            nc.vector.tensor_mul(pr3, lp_t[:, 0:SH], er_t[:, 1 : SH + 1])
            nc.vector.tensor_mul(pr4, lp_t[:, 0:SH], eer_t[:, 1 : SH + 1])
            for i, pr in ((1, pr2), (2, pr3), (3, pr4)):
                for w in range(SH // 512):
                    acc_mm((f"p{i}",), ps_vec[i], lhsT=ones_t,
                           rhs=pr[:, w * 512 : (w + 1) * 512],
                           last=w == SH // 512 - 1)

            # -------- dump psums --------
            dl_sb = acc.tile([64, 512], F32)
            nc.scalar.activation(dl_sb, ps_lse, AF.Copy)
            nc.sync.dma_start(out=dlse_d, in_=dl_sb)
            dx_sb = acc.tile([P, 8, P], F32)
            for q in range(2):
                nc.scalar.activation(dx_sb[:, q * 4 : (q + 1) * 4, :],
                                     ps_x[q][:, :, :], AF.Copy)
            nc.sync.dma_start(out=dx_d, in_=dx_sb)
            vec_sb = acc.tile([1, 4, 512], F32)
            for i in range(4):
                nc.scalar.activation(vec_sb[:, i, :], ps_vec[i], AF.Copy)
            nc.sync.dma_start(out=vec_d, in_=vec_sb)

    if compile:
        nc.compile()
    return nc


_program = None


def _get_program():
    global _program
    if _program is None:
        _program = build_program()
    return _program


def _pair_boundary(s):
    """The only clamped boundary pair term not covered on device:
    4 * [s[S-3]==1][s[S-2]==3][s[S-1]==2] per row."""
    m = (s[:, -3] == 1) & (s[:, -2] == 3) & (s[:, -1] == 2)
    return 4.0 * float(m.sum())


def combine_partials(results, s_full, ce_weights):
    """Host-side (float64) combination of per-core device partials."""
    w = np.asarray(ce_weights, np.float64)
    Wc = np.zeros(C, np.float64)
    Sc = np.zeros(C, np.float64)
    z0 = 0.0
    pen = 0.0
    r_idx = np.arange(64)
    p_idx = np.arange(P)
    for r in results:
        dl = r["diag_lse"].astype(np.float64)   # [64, 512]
        for c in range(C):
            Wc[c] += dl[r_idx, c * 64 + r_idx].sum()
        dx = r["diag_x"].astype(np.float64)     # [128, 8, 128]
        Sc += dx[p_idx, :, p_idx].sum(0)
        va = r["vec_acc"].astype(np.float64).reshape(4, 512)
        z0 += va[0].sum()
        pen += 2.0 * va[1].sum() + 3.0 * va[2].sum() + 4.0 * va[3].sum()
        sc = r["pen_scan"].astype(np.float64)
        pfa, mpa = sc[0:RB, 0], sc[0:RB, 1]
        pfb, mpb = sc[RB:P, 0], sc[RB:P, 1]
        pf = pfa + pfb
        mp = np.minimum(mpa, pfa + mpb)
        pen += (pf - 2.0 * np.minimum(0.0, mp)).sum()
    pen += _pair_boundary(s_full)
    ce_loss = float((w * (Wc - Sc)).sum()) / (B * S)
    nnz = B * S - z0
    penalty = pen / nnz
    return np.float32(ce_loss + PENALTY_WEIGHT * penalty)


def make_in_maps(logits, targets, predicted_structures):
    lg = np.ascontiguousarray(logits, dtype=np.float32)
    t = np.ascontiguousarray(targets, dtype=np.int32)
    s = np.ascontiguousarray(predicted_structures.reshape(B, S), dtype=np.int32)
    in_maps = []
    for core in range(NCORES):
        rows = slice(core * RB, (core + 1) * RB)
        in_maps.append({
            "logits": lg[rows].reshape(P, NP * C),
            "targets": t[rows].reshape(P, NP),
            "structs": s[rows],
        })
    return in_maps, s


def kernel(logits, targets, predicted_structures, ce_weights):
    in_maps, s = make_in_maps(logits, targets, predicted_structures)
    nc = _get_program()
    res = run_bass_kernel_spmd(nc, in_maps, core_ids=list(range(NCORES)))
    return combine_partials(res.results, s, ce_weights)



# revision 14
# speedup vs baseline: 1.3880x; 1.3880x over previous
"""Trainium2 Bass kernel for weighted-CE + structural-penalty loss (v2).

Full inputs -> data-parallel shard over batch across 8 NeuronCores ->
per-core Bass kernel computes small partial sums -> host combines in
float64.

Decomposition (per core, positions laid out [128 partitions, 2048]):
  ce_sum = sum_pos wt*lse - sum_c w_c * Sc
    wt   = w[t] (host-gathered fp16, 8-entry table lookup)
    lse  = ln(sum_c exp(x_c)): ACT Exp (fp16, class-planar layout from a
           host-side transpose) -> DVE pairwise adds -> ACT Ln
    A    = sum wt*lse: DVE mult + PE ones-colsum into PSUM [1,512]
    Sc   = sum_{t=c} x_c: 8 per-class is_equal masks (DVE tensor_scalar
           4x vs per-partition fp32 scalars), Z = m*x (one packed
           tensor_tensor), PE ones-colsums into per-class PSUM rows.
  penalty (per row, codes Q = s + 4*s[+1] + 16*s[+2] + 64*s[+3] packed
  host-side as fp16):
    bracket part: d = (Q%4==1) - (Q%4==2); pf = sum d (ts accum);
      H = relu-scan of d (tensor_tensor_scan add/max vs zeros); host
      chains row halves via U(seed) = relu(Hb - pfb - Ha).
    pair part: indicators (Q%16==9 | Q%64==45 | Q%256==189) as single
      two-op tensor_scalars, PE colsums with lhsT columns 2/3/4 into one
      PSUM [1,512]; host adds the tiny clamped-tail correction.
"""

import numpy as np

import concourse.bass as bass
import concourse.mybir as mybir
import concourse.tile as tile
from concourse import bacc
from concourse.bass_utils import run_bass_kernel_spmd

B, S, C = 512, 4096, 8
PENALTY_WEIGHT = 0.1
NCORES = 8
RB = B // NCORES          # rows (batch) per core
P = 128                   # SBUF partitions
NP = (RB * S) // P        # positions per partition (2048)
NCH = 4                   # CE chunks
PCH = NP // NCH           # positions per partition per chunk (512)

F32 = mybir.dt.float32
F16 = mybir.dt.float16
OP = mybir.AluOpType
AF = mybir.ActivationFunctionType


def _patch_act_tables():
    """Prefer the single table set containing Exp+Ln+Copy so the kernel
    pays one ACT_TABLE_LOAD instead of alternating per chunk."""
    import concourse.hw_specs as hw_specs
    if getattr(hw_specs, "_loss_kernel_tables_patched", False):
        return
    orig = hw_specs.get_activation_tables

    def patched(arch):
        t = orig(arch)
        pref = "natural_log_exp_and_others"
        if pref not in t:
            return t
        return {k: (v if k == pref else set()) for k, v in t.items()}

    hw_specs.get_activation_tables = patched
    bacc.get_activation_tables = patched
    hw_specs._loss_kernel_tables_patched = True


def build_program(compile=True):
    _patch_act_tables()
    nc = bacc.Bacc("TRN2", target_bir_lowering=False, debug=False)

    x_d = nc.dram_tensor("xpl", [P, NCH * C * PCH], F16, kind="ExternalInput").ap()
    wt_d = nc.dram_tensor("wt", [P, NP], F16, kind="ExternalInput").ap()
    wv_d = nc.dram_tensor("wvals", [P, C], F32, kind="ExternalInput").ap()
    # structure code streams: s, s+4*s1, s+4*s1+16*s2, s+4*s1+16*s2+64*s3
    # (single-op is_equal tests; mod/bitwise fail the DVE ISA check)
    q_d = [nc.dram_tensor(f"qc{j}", [P, NP], F16, kind="ExternalInput").ap()
           for j in range(4)]

    # cols: sum(lp), sum(rp), H, P2, P3, P4
    acc_d = nc.dram_tensor("accs", [P, 6], F32, kind="ExternalOutput").ap()
    # rows 0..7: per-class Sc colsums; row 8: A colsum
    sums_d = nc.dram_tensor("sums", [9, 512], F32, kind="ExternalOutput").ap()

    with tile.TileContext(nc) as tc:
        with (
            tc.tile_pool(name="const", bufs=1) as const,
            tc.tile_pool(name="xin", bufs=2) as xin,
            tc.tile_pool(name="ebuf", bufs=2) as ebuf,
            tc.tile_pool(name="mbuf", bufs=2) as mbuf,
            tc.tile_pool(name="zbuf", bufs=2) as zbuf,
            tc.tile_pool(name="mid", bufs=2) as mid,
            tc.tile_pool(name="pen", bufs=1) as pen,
            tc.tile_pool(name="acc", bufs=1) as acc,
            tc.tile_pool(name="psum", bufs=1, space="PSUM") as psum,
        ):
            # 3 colsum rows per PSUM bank (matmul outs must start at
            # partition 0/32/64): banks 0-2 hold [c0..c2], [c3..c5],
            # [c6, c7, A]; bank 3 holds the penalty colsum.
            ps_b = [psum.tile([65, 512], F32, name=f"ps_b{g}") for g in range(3)]

            def colsum_out(idx):
                return ps_b[idx // 3][(idx % 3) * 32 : (idx % 3) * 32 + 1, :]

            ones1 = const.tile([P, 1], F16)
            nc.vector.memset(ones1, 1.0)
            zer = const.tile([P, NP], F16)
            nc.vector.memset(zer, 0.0)

            wt_sb = const.tile([P, NP], F16)
            nc.sync.dma_start(out=wt_sb, in_=wt_d)
            wv_sb = const.tile([P, C], F32)
            nc.sync.dma_start(out=wv_sb, in_=wv_d)
            q_sb = [const.tile([P, NP], F16, name=f"q{j}") for j in range(4)]
            for j in range(4):
                nc.sync.dma_start(out=q_sb[j], in_=q_d[j])

            acc_sb = acc.tile([P, 6], F32)

            # ---------------- CE chunks ----------------
            for k in range(NCH):
                fl = k * C * PCH
                x_t = xin.tile([P, C, PCH], F16, tag="x")
                nc.sync.dma_start(out=x_t, in_=x_d[:, fl : fl + C * PCH])

                wtk = wt_sb[:, k * PCH : (k + 1) * PCH]

                # masks first (DVE) to overlap with exp (ACT)
                m_t = mbuf.tile([P, C, PCH], F16, tag="m")
                for c in range(C):
                    nc.vector.tensor_scalar(
                        out=m_t[:, c, :], in0=wtk,
                        scalar1=wv_sb[:, c : c + 1], scalar2=None,
                        op0=OP.is_equal)

                e_t = ebuf.tile([P, C, PCH], F16, tag="e")
                nc.scalar.activation(e_t, x_t, AF.Exp)

                z_t = zbuf.tile([P, C, PCH], F16, tag="z")
                nc.vector.tensor_mul(z_t, m_t, x_t)

                t4 = mid.tile([P, 4, PCH], F16, tag="t4")
                nc.vector.tensor_add(t4, e_t[:, 0:4, :], e_t[:, 4:8, :])
                t2 = mid.tile([P, 2, PCH], F16, tag="t2")
                nc.vector.tensor_add(t2, t4[:, 0:2, :], t4[:, 2:4, :])
                se = mid.tile([P, PCH], F16, tag="se")
                nc.vector.tensor_add(se, t2[:, 0, :], t2[:, 1, :])
                lse = mid.tile([P, PCH], F16, tag="lse")
                nc.scalar.activation(lse, se, AF.Ln)
                prod = mid.tile([P, PCH], F16, tag="prod")
                nc.vector.tensor_mul(prod, wtk, lse)

                last = k == NCH - 1
                nc.tensor.matmul(colsum_out(8), lhsT=ones1, rhs=prod,
                                 start=(k == 0), stop=last)
                for c in range(C):
                    nc.tensor.matmul(colsum_out(c), lhsT=ones1,
                                     rhs=z_t[:, c, :],
                                     start=(k == 0), stop=last)

            # ---------------- penalty ----------------
            lp_t = pen.tile([P, NP], F16)
            nc.vector.tensor_scalar(out=lp_t, in0=q_sb[0], scalar1=1.0,
                                    scalar2=None, op0=OP.is_equal, op1=OP.add,
                                    accum_out=acc_sb[:, 0:1])
            rp_t = pen.tile([P, NP], F16)
            nc.vector.tensor_scalar(out=rp_t, in0=q_sb[0], scalar1=2.0,
                                    scalar2=None, op0=OP.is_equal, op1=OP.add,
                                    accum_out=acc_sb[:, 1:2])
            d_t = pen.tile([P, NP], F16)
            nc.vector.tensor_sub(d_t, lp_t, rp_t)
            h_t = pen.tile([P, NP], F16)
            nc.vector.tensor_tensor_scan(out=h_t, data0=d_t, data1=zer,
                                         initial=0.0, op0=OP.add, op1=OP.max)
            nc.vector.tensor_copy(out=acc_sb[:, 2:3], in_=h_t[:, NP - 1 : NP])

            pr_t = [pen.tile([P, NP], F16, name=f"pr{i}") for i in range(3)]
            for i, tv in enumerate((9.0, 45.0, 189.0)):
                nc.vector.tensor_scalar(out=pr_t[i], in0=q_sb[i + 1],
                                        scalar1=tv, scalar2=None,
                                        op0=OP.is_equal, op1=OP.add,
                                        accum_out=acc_sb[:, 3 + i : 4 + i])

            # ---------------- dumps ----------------
            nc.sync.dma_start(out=acc_d, in_=acc_sb)
            zs = [acc.tile([65, 512], F32, name=f"zs{g}") for g in range(3)]
            for g in range(3):
                nc.scalar.activation(zs[g], ps_b[g], AF.Copy)
                rows = bass.AP(tensor=zs[g].tensor, offset=zs[g].offset,
                               ap=[[zs[g].ap[0][0] * 32, 3], [1, 512]])
                nc.sync.dma_start(out=sums_d[g * 3 : g * 3 + 3, :], in_=rows)

    if compile:
        nc.compile()
    return nc


_program = None


def _get_program():
    global _program
    if _program is None:
        _program = build_program()
    return _program


def _unique_fp16(w):
    """fp16 weights, nudged to pairwise-distinct bit patterns."""
    wq = w.astype(np.float16)
    seen = set()
    for i in range(wq.shape[0]):
        v = wq[i]
        while v.tobytes() in seen:
            v = np.nextafter(v, np.float16(2.0), dtype=np.float16)
        seen.add(v.tobytes())
        wq[i] = v
    return wq


def make_in_maps(logits, targets, predicted_structures, ce_weights):
    t = np.asarray(targets, dtype=np.int64)
    s = np.asarray(predicted_structures).reshape(B, S).astype(np.int64)
    lg = np.asarray(logits, dtype=np.float32)
    wq = _unique_fp16(np.asarray(ce_weights, dtype=np.float64))
    wv32 = np.ascontiguousarray(
        np.broadcast_to(wq.astype(np.float32), (P, C)))

    i = np.arange(S)
    s1 = s[:, np.minimum(i + 1, S - 1)]
    s2 = s[:, np.minimum(i + 2, S - 1)]
    s3 = s[:, np.minimum(i + 3, S - 1)]
    qs = [s, s + 4 * s1, s + 4 * s1 + 16 * s2,
          s + 4 * s1 + 16 * s2 + 64 * s3]

    def split(a, dt):
        return np.ascontiguousarray(
            a.reshape(RB, 2, NP).transpose(1, 0, 2).reshape(P, NP)).astype(dt)

    in_maps = []
    for core in range(NCORES):
        rows = slice(core * RB, (core + 1) * RB)
        # row r -> partition r (pos 0..2047) and 64+r (pos 2048..4095)
        x_pp = lg[rows].reshape(RB, 2, NP, C).transpose(1, 0, 2, 3)
        x16 = np.ascontiguousarray(
            x_pp.reshape(P, NCH, PCH, C).transpose(0, 1, 3, 2)
        ).astype(np.float16).reshape(P, NCH * C * PCH)
        wt16 = np.ascontiguousarray(wq[split(t[rows], np.int64)])
        m = {"xpl": x16, "wt": wt16, "wvals": wv32}
        for j in range(4):
            m[f"qc{j}"] = split(qs[j][rows], np.float16)
        in_maps.append(m)
    return in_maps, t, s, wq


def combine_partials(results, t, s, ce_weights):
    w = np.asarray(ce_weights, np.float64)
    A = 0.0
    Sc = np.zeros(C, np.float64)
    pen = 0.0
    for r in results:
        sums = r["sums"].astype(np.float64)
        A += sums[8].sum()
        Sc += sums[0:8].sum(axis=1)
        accs = r["accs"].astype(np.float64)
        pen += (2.0 * accs[:, 3] + 3.0 * accs[:, 4] + 4.0 * accs[:, 5]).sum()
        pf = accs[:, 0] - accs[:, 1]
        pfa, ha = pf[0:RB], accs[0:RB, 2]
        pfb, hb = pf[RB:P], accs[RB:P, 2]
        ua = ha - pfa
        ub = np.maximum(hb - pfb - ha, 0.0)
        pen += ((pfa + pfb) + 2.0 * (ua + ub)).sum()

    # clamped-tail correction for pair3/pair4 (reference clamps dot offsets
    # at S-2; the device codes clamp uniformly at S-1)
    i = np.arange(S - 4, S)
    d1r = s[:, np.minimum(i + 1, S - 2)]
    d2r = s[:, np.minimum(i + 2, S - 2)]
    r1 = s[:, np.minimum(i + 1, S - 1)]
    r2 = s[:, np.minimum(i + 2, S - 1)]
    r3 = s[:, np.minimum(i + 3, S - 1)]
    lp = s[:, i] == 1
    ref_p3 = lp & (d1r == 3) & (r2 == 2)
    ref_p4 = lp & (d1r == 3) & (d2r == 3) & (r3 == 2)
    dev_p3 = lp & (r1 == 3) & (r2 == 2)
    dev_p4 = lp & (r1 == 3) & (r2 == 3) & (r3 == 2)
    pen += (3.0 * (ref_p3.astype(np.float64) - dev_p3)
            + 4.0 * (ref_p4.astype(np.float64) - dev_p4)).sum()

    nnz = float((t != 0).sum())
    ce = (A - (w * Sc).sum()) / (B * S)
    penalty = pen / nnz
    return np.float32(ce + PENALTY_WEIGHT * penalty)


def kernel(logits, targets, predicted_structures, ce_weights):
    in_maps, t, s, wq = make_in_maps(
        logits, targets, predicted_structures, ce_weights)
    nc = _get_program()
    res = run_bass_kernel_spmd(nc, in_maps, core_ids=list(range(NCORES)))
    return combine_partials(res.results, t, s, ce_weights)


# revision 21
# speedup vs baseline: 1.6274x; 1.1725x over previous
"""Trainium2 Bass kernel for weighted-CE + structural-penalty loss (v2).

Full inputs -> data-parallel shard over batch across 8 NeuronCores ->
per-core Bass kernel computes small partial sums -> host combines in
float64.

Decomposition (per core, positions laid out [128 partitions, 2048]):
  ce_sum = sum_pos wt*lse - sum_c w_c * Sc
    wt   = w[t] (host-gathered fp16, 8-entry table lookup)
    lse  = ln(sum_c exp(x_c)): ACT Exp (fp16, class-planar layout from a
           host-side transpose) -> DVE pairwise adds -> ACT Ln
    A    = sum wt*lse: DVE mult + PE ones-colsum into PSUM [1,512]
    Sc   = sum_{t=c} x_c: 8 per-class is_equal masks (DVE tensor_scalar
           4x vs per-partition fp32 scalars), Z = m*x (one packed
           tensor_tensor), PE ones-colsums into per-class PSUM rows.
  penalty (per row, codes Q = s + 4*s[+1] + 16*s[+2] + 64*s[+3] packed
  host-side as fp16):
    bracket part: d = (Q%4==1) - (Q%4==2); pf = sum d (ts accum);
      H = relu-scan of d (tensor_tensor_scan add/max vs zeros); host
      chains row halves via U(seed) = relu(Hb - pfb - Ha).
    pair part: indicators (Q%16==9 | Q%64==45 | Q%256==189) as single
      two-op tensor_scalars, PE colsums with lhsT columns 2/3/4 into one
      PSUM [1,512]; host adds the tiny clamped-tail correction.
"""

import numpy as np

import concourse.bass as bass
import concourse.mybir as mybir
import concourse.tile as tile
from concourse import bacc
from concourse.bass_utils import run_bass_kernel_spmd

B, S, C = 512, 4096, 8
PENALTY_WEIGHT = 0.1
NCORES = 8
RB = B // NCORES          # rows (batch) per core
P = 128                   # SBUF partitions
NP = (RB * S) // P        # positions per partition (2048)
NCH = 4                   # CE chunks
PCH = NP // NCH           # positions per partition per chunk (512)

F32 = mybir.dt.float32
F16 = mybir.dt.float16
OP = mybir.AluOpType
AF = mybir.ActivationFunctionType


def _patch_act_tables():
    """Prefer the single table set containing Exp+Ln+Copy so the kernel
    pays one ACT_TABLE_LOAD instead of alternating per chunk."""
    import concourse.hw_specs as hw_specs
    if getattr(hw_specs, "_loss_kernel_tables_patched", False):
        return
    orig = hw_specs.get_activation_tables

    def patched(arch):
        t = orig(arch)
        pref = "natural_log_exp_and_others"
        if pref not in t:
            return t
        return {k: (v if k == pref else set()) for k, v in t.items()}

    hw_specs.get_activation_tables = patched
    bacc.get_activation_tables = patched
    hw_specs._loss_kernel_tables_patched = True


def build_program(compile=True):
    _patch_act_tables()
    nc = bacc.Bacc("TRN2", target_bir_lowering=False, debug=False)

    x_d = nc.dram_tensor("xpl", [P, NCH * C * PCH], F16, kind="ExternalInput").ap()
    wt_d = nc.dram_tensor("wt", [P, NP], F16, kind="ExternalInput").ap()
    wv_d = nc.dram_tensor("wvals", [P, C], F32, kind="ExternalInput").ap()
    # structure code streams: s, s+4*s1, s+4*s1+16*s2, s+4*s1+16*s2+64*s3
    # (single-op is_equal tests; mod/bitwise fail the DVE ISA check)
    q_d = [nc.dram_tensor(f"qc{j}", [P, NP], F16, kind="ExternalInput").ap()
           for j in range(4)]

    # cols: pf (= sum d), H
    acc_d = nc.dram_tensor("accs", [P, 2], F32, kind="ExternalOutput").ap()
    # rows 0..7: per-class Sc colsums; row 8: A colsum; row 9: pen colsum
    sums_d = nc.dram_tensor("sums", [10, 512], F32, kind="ExternalOutput").ap()

    with tile.TileContext(nc) as tc:
        with (
            tc.tile_pool(name="const", bufs=1) as const,
            tc.tile_pool(name="xin", bufs=4) as xin,
            tc.tile_pool(name="ebuf", bufs=2) as ebuf,
            tc.tile_pool(name="mbuf", bufs=2) as mbuf,
            tc.tile_pool(name="zbuf", bufs=2) as zbuf,
            tc.tile_pool(name="mid", bufs=2) as mid,
            tc.tile_pool(name="pen", bufs=1) as pen,
            tc.tile_pool(name="acc", bufs=1) as acc,
            tc.tile_pool(name="psum", bufs=1, space="PSUM") as psum,
        ):
            # 3 colsum rows per PSUM bank (matmul outs must start at
            # partition 0/32/64): banks 0-2 hold [c0..c2], [c3..c5],
            # [c6, c7, A]; bank 3 holds the penalty colsum.
            ps_b = [psum.tile([65, 512], F32, name=f"ps_b{g}") for g in range(3)]
            ps_p = psum.tile([1, 512], F32, name="ps_p")

            def colsum_out(idx):
                return ps_b[idx // 3][(idx % 3) * 32 : (idx % 3) * 32 + 1, :]

            ones1 = const.tile([P, 1], F16)
            nc.vector.memset(ones1, 1.0)
            penw = const.tile([P, 3], F16)
            for j, v in enumerate((2.0, 3.0, 4.0)):
                nc.vector.memset(penw[:, j : j + 1], v)
            zer = const.tile([P, NP], F16)
            nc.vector.memset(zer, 0.0)

            # DMA order: wt/wv first (masks need them), then the x chunks
            # (the pipeline), penalty code streams last.
            wt_sb = const.tile([P, NP], F16)
            nc.sync.dma_start(out=wt_sb, in_=wt_d)
            wv_sb = const.tile([P, C], F32)
            nc.sync.dma_start(out=wv_sb, in_=wv_d)

            acc_sb = acc.tile([P, 2], F32)

            # full-width per-class masks, up front (one DVE op per class)
            m_full = const.tile([P, C, NP], F16)
            for c in range(C):
                nc.vector.tensor_scalar(
                    out=m_full[:, c, :], in0=wt_sb,
                    scalar1=wv_sb[:, c : c + 1], scalar2=None,
                    op0=OP.is_equal)

            # ---------------- CE chunks ----------------
            x_ts = []
            for k in range(NCH):
                fl = k * C * PCH
                x_t = xin.tile([P, C, PCH], F16, tag="x")
                nc.sync.dma_start(out=x_t, in_=x_d[:, fl : fl + C * PCH])
                x_ts.append(x_t)

            q_sb = [const.tile([P, NP], F16, name=f"q{j}") for j in range(4)]
            for j in range(4):
                nc.sync.dma_start(out=q_sb[j], in_=q_d[j])

            for k in range(NCH):
                x_t = x_ts[k]
                wtk = wt_sb[:, k * PCH : (k + 1) * PCH]

                e_t = ebuf.tile([P, C, PCH], F16, tag="e")
                nc.scalar.activation(e_t, x_t, AF.Exp)

                z_t = zbuf.tile([P, C, PCH], F16, tag="z")
                nc.vector.tensor_mul(
                    z_t, m_full[:, :, k * PCH : (k + 1) * PCH], x_t)

                t4 = mid.tile([P, 4, PCH], F16, tag="t4")
                nc.vector.tensor_add(t4, e_t[:, 0:4, :], e_t[:, 4:8, :])
                t2 = mid.tile([P, 2, PCH], F16, tag="t2")
                nc.vector.tensor_add(t2, t4[:, 0:2, :], t4[:, 2:4, :])
                se = mid.tile([P, PCH], F16, tag="se")
                nc.vector.tensor_add(se, t2[:, 0, :], t2[:, 1, :])
                lse = mid.tile([P, PCH], F16, tag="lse")
                nc.scalar.activation(lse, se, AF.Ln)
                prod = mid.tile([P, PCH], F16, tag="prod")
                nc.vector.tensor_mul(prod, wtk, lse)

                last = k == NCH - 1
                nc.tensor.matmul(colsum_out(8), lhsT=ones1, rhs=prod,
                                 start=(k == 0), stop=last)
                for c in range(C):
                    nc.tensor.matmul(colsum_out(c), lhsT=ones1,
                                     rhs=z_t[:, c, :],
                                     start=(k == 0), stop=last)

            # ---------------- penalty ----------------
            lp_t = pen.tile([P, NP], F16)
            nc.vector.tensor_scalar(out=lp_t, in0=q_sb[0], scalar1=1.0,
                                    scalar2=None, op0=OP.is_equal)
            rp_t = pen.tile([P, NP], F16)
            nc.vector.tensor_scalar(out=rp_t, in0=q_sb[0], scalar1=2.0,
                                    scalar2=None, op0=OP.is_equal)
            d_t = pen.tile([P, NP], F16)
            nc.vector.tensor_sub(d_t, lp_t, rp_t)
            # pf = sum d via the ACT accumulator (keeps it off the DVE)
            djunk = pen.tile([P, NP], F16)
            nc.scalar.activation(djunk, d_t, AF.Copy,
                                 accum_out=acc_sb[:, 0:1])
            h_t = pen.tile([P, NP], F16)
            nc.vector.tensor_tensor_scan(out=h_t, data0=d_t, data1=zer,
                                         initial=0.0, op0=OP.add, op1=OP.max)
            nc.vector.tensor_copy(out=acc_sb[:, 1:2], in_=h_t[:, NP - 1 : NP])

            pr_t = [pen.tile([P, NP], F16, name=f"pr{i}") for i in range(3)]
            for i, tv in enumerate((9.0, 45.0, 189.0)):
                nc.vector.tensor_scalar(out=pr_t[i], in0=q_sb[i + 1],
                                        scalar1=tv, scalar2=None,
                                        op0=OP.is_equal)
            nmm = NP // 512
            for i in range(3):
                for w in range(nmm):
                    nc.tensor.matmul(ps_p, lhsT=penw[:, i : i + 1],
                                     rhs=pr_t[i][:, w * 512 : (w + 1) * 512],
                                     start=(i == 0 and w == 0),
                                     stop=(i == 2 and w == nmm - 1))

            # ---------------- dumps ----------------
            nc.sync.dma_start(out=acc_d, in_=acc_sb)
            zs = [acc.tile([65, 512], F32, name=f"zs{g}") for g in range(3)]
            for g in range(3):
                nc.scalar.activation(zs[g], ps_b[g], AF.Copy)
                rows = bass.AP(tensor=zs[g].tensor, offset=zs[g].offset,
                               ap=[[zs[g].ap[0][0] * 32, 3], [1, 512]])
                nc.sync.dma_start(out=sums_d[g * 3 : g * 3 + 3, :], in_=rows)
            psb = acc.tile([1, 512], F32)
            nc.scalar.activation(psb, ps_p, AF.Copy)
            nc.sync.dma_start(out=sums_d[9:10, :], in_=psb)

    if compile:
        nc.compile()
    return nc


_program = None


def _get_program():
    global _program
    if _program is None:
        _program = build_program()
    return _program


def _unique_fp16(w):
    """fp16 weights, nudged to pairwise-distinct bit patterns."""
    wq = w.astype(np.float16)
    seen = set()
    for i in range(wq.shape[0]):
        v = wq[i]
        while v.tobytes() in seen:
            v = np.nextafter(v, np.float16(2.0), dtype=np.float16)
        seen.add(v.tobytes())
        wq[i] = v
    return wq


def make_in_maps(logits, targets, predicted_structures, ce_weights):
    t = np.asarray(targets, dtype=np.int64)
    s = np.asarray(predicted_structures).reshape(B, S).astype(np.int64)
    lg = np.asarray(logits, dtype=np.float32)
    wq = _unique_fp16(np.asarray(ce_weights, dtype=np.float64))
    wv32 = np.ascontiguousarray(
        np.broadcast_to(wq.astype(np.float32), (P, C)))

    i = np.arange(S)
    s1 = s[:, np.minimum(i + 1, S - 1)]
    s2 = s[:, np.minimum(i + 2, S - 1)]
    s3 = s[:, np.minimum(i + 3, S - 1)]
    qs = [s, s + 4 * s1, s + 4 * s1 + 16 * s2,
          s + 4 * s1 + 16 * s2 + 64 * s3]

    def split(a, dt):
        return np.ascontiguousarray(
            a.reshape(RB, 2, NP).transpose(1, 0, 2).reshape(P, NP)).astype(dt)

    in_maps = []
    for core in range(NCORES):
        rows = slice(core * RB, (core + 1) * RB)
        # row r -> partition r (pos 0..2047) and 64+r (pos 2048..4095)
        x_pp = lg[rows].reshape(RB, 2, NP, C).transpose(1, 0, 2, 3)
        x16 = np.ascontiguousarray(
            x_pp.reshape(P, NCH, PCH, C).transpose(0, 1, 3, 2)
        ).astype(np.float16).reshape(P, NCH * C * PCH)
        wt16 = np.ascontiguousarray(wq[split(t[rows], np.int64)])
        m = {"xpl": x16, "wt": wt16, "wvals": wv32}
        for j in range(4):
            m[f"qc{j}"] = split(qs[j][rows], np.float16)
        in_maps.append(m)
    return in_maps, t, s, wq


def combine_partials(results, t, s, ce_weights):
    w = np.asarray(ce_weights, np.float64)
    A = 0.0
    Sc = np.zeros(C, np.float64)
    pen = 0.0
    for r in results:
        sums = r["sums"].astype(np.float64)
        A += sums[8].sum()
        Sc += sums[0:8].sum(axis=1)
        pen += sums[9].sum()
        accs = r["accs"].astype(np.float64)
        pfa, ha = accs[0:RB, 0], accs[0:RB, 1]
        pfb, hb = accs[RB:P, 0], accs[RB:P, 1]
        ua = ha - pfa
        ub = np.maximum(hb - pfb - ha, 0.0)
        pen += ((pfa + pfb) + 2.0 * (ua + ub)).sum()

    # clamped-tail correction for pair3/pair4 (reference clamps dot offsets
    # at S-2; the device codes clamp uniformly at S-1)
    i = np.arange(S - 4, S)
    d1r = s[:, np.minimum(i + 1, S - 2)]
    d2r = s[:, np.minimum(i + 2, S - 2)]
    r1 = s[:, np.minimum(i + 1, S - 1)]
    r2 = s[:, np.minimum(i + 2, S - 1)]
    r3 = s[:, np.minimum(i + 3, S - 1)]
    lp = s[:, i] == 1
    ref_p3 = lp & (d1r == 3) & (r2 == 2)
    ref_p4 = lp & (d1r == 3) & (d2r == 3) & (r3 == 2)
    dev_p3 = lp & (r1 == 3) & (r2 == 2)
    dev_p4 = lp & (r1 == 3) & (r2 == 3) & (r3 == 2)
    pen += (3.0 * (ref_p3.astype(np.float64) - dev_p3)
            + 4.0 * (ref_p4.astype(np.float64) - dev_p4)).sum()

    nnz = float((t != 0).sum())
    ce = (A - (w * Sc).sum()) / (B * S)
    penalty = pen / nnz
    return np.float32(ce + PENALTY_WEIGHT * penalty)


def kernel(logits, targets, predicted_structures, ce_weights):
    in_maps, t, s, wq = make_in_maps(
        logits, targets, predicted_structures, ce_weights)
    nc = _get_program()
    res = run_bass_kernel_spmd(nc, in_maps, core_ids=list(range(NCORES)))
    return combine_partials(res.results, t, s, ce_weights)


# revision 25
# speedup vs baseline: 1.6892x; 1.0380x over previous
"""Trainium2 Bass kernel for weighted-CE + structural-penalty loss (v2).

Full inputs -> data-parallel shard over batch across 8 NeuronCores ->
per-core Bass kernel computes small partial sums -> host combines in
float64.

Decomposition (per core, positions laid out [128 partitions, 2048]):
  ce_sum = sum_pos wt*lse - sum_c w_c * Sc
    wt   = w[t] (host-gathered fp16, 8-entry table lookup)
    lse  = ln(sum_c exp(x_c)): ACT Exp (fp16, class-planar layout from a
           host-side transpose) -> DVE pairwise adds -> ACT Ln
    A    = sum wt*lse: DVE mult + PE ones-colsum into PSUM [1,512]
    Sc   = sum_{t=c} x_c: 8 per-class is_equal masks (DVE tensor_scalar
           4x vs per-partition fp32 scalars), Z = m*x (one packed
           tensor_tensor), PE ones-colsums into per-class PSUM rows.
  penalty (per row, codes Q = s + 4*s[+1] + 16*s[+2] + 64*s[+3] packed
  host-side as fp16):
    bracket part: d = (Q%4==1) - (Q%4==2); pf = sum d (ts accum);
      H = relu-scan of d (tensor_tensor_scan add/max vs zeros); host
      chains row halves via U(seed) = relu(Hb - pfb - Ha).
    pair part: indicators (Q%16==9 | Q%64==45 | Q%256==189) as single
      two-op tensor_scalars, PE colsums with lhsT columns 2/3/4 into one
      PSUM [1,512]; host adds the tiny clamped-tail correction.
"""

import numpy as np

import concourse.bass as bass
import concourse.mybir as mybir
import concourse.tile as tile
from concourse import bacc
from concourse.bass_utils import run_bass_kernel_spmd

B, S, C = 512, 4096, 8
PENALTY_WEIGHT = 0.1
NCORES = 8
RB = B // NCORES          # rows (batch) per core
P = 128                   # SBUF partitions
NP = (RB * S) // P        # positions per partition (2048)
NCH = 4                   # CE chunks
PCH = NP // NCH           # positions per partition per chunk (512)

F32 = mybir.dt.float32
F16 = mybir.dt.float16
OP = mybir.AluOpType
AF = mybir.ActivationFunctionType


def _patch_act_tables():
    """Prefer the single table set containing Exp+Ln+Copy so the kernel
    pays one ACT_TABLE_LOAD instead of alternating per chunk."""
    import concourse.hw_specs as hw_specs
    if getattr(hw_specs, "_loss_kernel_tables_patched", False):
        return
    orig = hw_specs.get_activation_tables

    def patched(arch):
        t = orig(arch)
        pref = "natural_log_exp_and_others"
        if pref not in t:
            return t
        return {k: (v if k == pref else set()) for k, v in t.items()}

    hw_specs.get_activation_tables = patched
    bacc.get_activation_tables = patched
    hw_specs._loss_kernel_tables_patched = True


def build_program(compile=True):
    _patch_act_tables()
    nc = bacc.Bacc("TRN2", target_bir_lowering=False, debug=False)

    x_d = nc.dram_tensor("xpl", [P, NCH * C * PCH], F16, kind="ExternalInput").ap()
    wt_d = nc.dram_tensor("wt", [P, NP], F16, kind="ExternalInput").ap()
    wv_d = nc.dram_tensor("wvals", [P, C], F32, kind="ExternalInput").ap()
    # structure code streams: s, s+4*s1, s+4*s1+16*s2, s+4*s1+16*s2+64*s3
    # (single-op is_equal tests; mod/bitwise fail the DVE ISA check)
    q_d = [nc.dram_tensor(f"qc{j}", [P, NP], F16, kind="ExternalInput").ap()
           for j in range(4)]

    # cols: pf (= sum d), H
    acc_d = nc.dram_tensor("accs", [P, 2], F32, kind="ExternalOutput").ap()
    # rows 0..7: per-class Sc colsums; row 8: A colsum; row 9: pen colsum
    sums_d = nc.dram_tensor("sums", [10, 512], F32, kind="ExternalOutput").ap()

    with tile.TileContext(nc) as tc:
        with (
            tc.tile_pool(name="const", bufs=1) as const,
            tc.tile_pool(name="xin", bufs=4) as xin,
            tc.tile_pool(name="ebuf", bufs=2) as ebuf,
            tc.tile_pool(name="mbuf", bufs=2) as mbuf,
            tc.tile_pool(name="zbuf", bufs=2) as zbuf,
            tc.tile_pool(name="mid", bufs=2) as mid,
            tc.tile_pool(name="pen", bufs=1) as pen,
            tc.tile_pool(name="acc", bufs=1) as acc,
            tc.tile_pool(name="psum", bufs=1, space="PSUM") as psum,
        ):
            # 3 colsum rows per PSUM bank (matmul outs must start at
            # partition 0/32/64): banks 0-2 hold [c0..c2], [c3..c5],
            # [c6, c7, A]; bank 3 holds the penalty colsum.
            ps_b = [psum.tile([65, 512], F32, name=f"ps_b{g}") for g in range(3)]
            ps_p = psum.tile([1, 512], F32, name="ps_p")

            def colsum_out(idx):
                return ps_b[idx // 3][(idx % 3) * 32 : (idx % 3) * 32 + 1, :]

            ones1 = const.tile([P, 1], F16)
            nc.gpsimd.memset(ones1, 1.0)
            penw = const.tile([P, 3], F16)
            for j, v in enumerate((2.0, 3.0, 4.0)):
                nc.gpsimd.memset(penw[:, j : j + 1], v)
            zer = const.tile([P, NP], F16)
            nc.gpsimd.memset(zer, 0.0)

            # DMA order: wt/wv first (masks need them), then the x chunks
            # (the pipeline), penalty code streams last.
            wt_sb = const.tile([P, NP], F16)
            nc.sync.dma_start(out=wt_sb, in_=wt_d)
            wv_sb = const.tile([P, C], F32)
            nc.sync.dma_start(out=wv_sb, in_=wv_d)

            acc_sb = acc.tile([P, 2], F32)

            # full-width per-class masks, up front (one DVE op per class)
            m_full = const.tile([P, C, NP], F16)
            for c in range(C):
                nc.vector.tensor_scalar(
                    out=m_full[:, c, :], in0=wt_sb,
                    scalar1=wv_sb[:, c : c + 1], scalar2=None,
                    op0=OP.is_equal)

            # ---------------- CE chunks ----------------
            # interleave x-chunk and penalty-code DMAs so penalty work can
            # fill DVE gaps instead of piling up at the end
            x_ts = []
            q_sb = [const.tile([P, NP], F16, name=f"q{j}") for j in range(4)]
            for k in range(NCH):
                fl = k * C * PCH
                x_t = xin.tile([P, C, PCH], F16, tag="x")
                nc.sync.dma_start(out=x_t, in_=x_d[:, fl : fl + C * PCH])
                x_ts.append(x_t)
                nc.sync.dma_start(out=q_sb[k], in_=q_d[k])

            nmm = NP // 512

            def pen_stream0():
                # lp/rp/d on DVE; pf accum on ACT; the relu-scan on gpsimd
                lp_t = pen.tile([P, NP], F16)
                nc.vector.tensor_scalar(out=lp_t, in0=q_sb[0], scalar1=1.0,
                                        scalar2=None, op0=OP.is_equal)
                rp_t = pen.tile([P, NP], F16)
                nc.vector.tensor_scalar(out=rp_t, in0=q_sb[0], scalar1=2.0,
                                        scalar2=None, op0=OP.is_equal)
                d_t = pen.tile([P, NP], F16)
                nc.vector.tensor_sub(d_t, lp_t, rp_t)
                djunk = pen.tile([P, NP], F16)
                nc.scalar.activation(djunk, d_t, AF.Copy,
                                     accum_out=acc_sb[:, 0:1])
                h_t = pen.tile([P, NP], F16)
                nc.vector.tensor_tensor_scan(out=h_t, data0=d_t, data1=zer,
                                             initial=0.0, op0=OP.add,
                                             op1=OP.max)
                nc.vector.tensor_copy(out=acc_sb[:, 1:2],
                                      in_=h_t[:, NP - 1 : NP])

            def pen_pairs(i, tv):
                pr = pen.tile([P, NP], F16, name=f"pr{i}")
                nc.vector.tensor_scalar(out=pr, in0=q_sb[i + 1], scalar1=tv,
                                        scalar2=None, op0=OP.is_equal)
                for w in range(nmm):
                    nc.tensor.matmul(ps_p, lhsT=penw[:, i : i + 1],
                                     rhs=pr[:, w * 512 : (w + 1) * 512],
                                     start=(i == 0 and w == 0),
                                     stop=(i == 2 and w == nmm - 1))

            pen_work = [pen_stream0,
                        lambda: pen_pairs(0, 9.0),
                        lambda: pen_pairs(1, 45.0),
                        lambda: pen_pairs(2, 189.0)]

            for k in range(NCH):
                x_t = x_ts[k]
                wtk = wt_sb[:, k * PCH : (k + 1) * PCH]

                e_t = ebuf.tile([P, C, PCH], F16, tag="e")
                nc.scalar.activation(e_t, x_t, AF.Exp)

                z_t = zbuf.tile([P, C, PCH], F16, tag="z")
                nc.vector.tensor_mul(
                    z_t, m_full[:, :, k * PCH : (k + 1) * PCH], x_t)

                t4 = mid.tile([P, 4, PCH], F16, tag="t4")
                nc.vector.tensor_add(t4, e_t[:, 0:4, :], e_t[:, 4:8, :])
                t2 = mid.tile([P, 2, PCH], F16, tag="t2")
                nc.vector.tensor_add(t2, t4[:, 0:2, :], t4[:, 2:4, :])
                se = mid.tile([P, PCH], F16, tag="se")
                nc.vector.tensor_add(se, t2[:, 0, :], t2[:, 1, :])
                lse = mid.tile([P, PCH], F16, tag="lse")
                nc.scalar.activation(lse, se, AF.Ln)
                prod = mid.tile([P, PCH], F16, tag="prod")
                nc.vector.tensor_mul(prod, wtk, lse)

                last = k == NCH - 1
                nc.tensor.matmul(colsum_out(8), lhsT=ones1, rhs=prod,
                                 start=(k == 0), stop=last)
                for c in range(C):
                    nc.tensor.matmul(colsum_out(c), lhsT=ones1,
                                     rhs=z_t[:, c, :],
                                     start=(k == 0), stop=last)
                pen_work[k]()

            # ---------------- dumps ----------------
            nc.sync.dma_start(out=acc_d, in_=acc_sb)
            zs = [acc.tile([65, 512], F32, name=f"zs{g}") for g in range(3)]
            for g in range(3):
                nc.scalar.activation(zs[g], ps_b[g], AF.Copy)
                rows = bass.AP(tensor=zs[g].tensor, offset=zs[g].offset,
                               ap=[[zs[g].ap[0][0] * 32, 3], [1, 512]])
                nc.sync.dma_start(out=sums_d[g * 3 : g * 3 + 3, :], in_=rows)
            psb = acc.tile([1, 512], F32)
            nc.scalar.activation(psb, ps_p, AF.Copy)
            nc.sync.dma_start(out=sums_d[9:10, :], in_=psb)

    if compile:
        nc.compile()
    return nc


_program = None


def _get_program():
    global _program
    if _program is None:
        _program = build_program()
    return _program


def _unique_fp16(w):
    """fp16 weights, nudged to pairwise-distinct bit patterns."""
    wq = w.astype(np.float16)
    seen = set()
    for i in range(wq.shape[0]):
        v = wq[i]
        while v.tobytes() in seen:
            v = np.nextafter(v, np.float16(2.0), dtype=np.float16)
        seen.add(v.tobytes())
        wq[i] = v
    return wq


def make_in_maps(logits, targets, predicted_structures, ce_weights):
    t = np.asarray(targets, dtype=np.int64)
    s = np.asarray(predicted_structures).reshape(B, S).astype(np.int64)
    lg = np.asarray(logits, dtype=np.float32)
    wq = _unique_fp16(np.asarray(ce_weights, dtype=np.float64))
    wv32 = np.ascontiguousarray(
        np.broadcast_to(wq.astype(np.float32), (P, C)))

    i = np.arange(S)
    s1 = s[:, np.minimum(i + 1, S - 1)]
    s2 = s[:, np.minimum(i + 2, S - 1)]
    s3 = s[:, np.minimum(i + 3, S - 1)]
    qs = [s, s + 4 * s1, s + 4 * s1 + 16 * s2,
          s + 4 * s1 + 16 * s2 + 64 * s3]

    def split(a, dt):
        return np.ascontiguousarray(
            a.reshape(RB, 2, NP).transpose(1, 0, 2).reshape(P, NP)).astype(dt)

    in_maps = []
    for core in range(NCORES):
        rows = slice(core * RB, (core + 1) * RB)
        # row r -> partition r (pos 0..2047) and 64+r (pos 2048..4095)
        x_pp = lg[rows].reshape(RB, 2, NP, C).transpose(1, 0, 2, 3)
        x16 = np.ascontiguousarray(
            x_pp.reshape(P, NCH, PCH, C).transpose(0, 1, 3, 2)
        ).astype(np.float16).reshape(P, NCH * C * PCH)
        wt16 = np.ascontiguousarray(wq[split(t[rows], np.int64)])
        m = {"xpl": x16, "wt": wt16, "wvals": wv32}
        for j in range(4):
            m[f"qc{j}"] = split(qs[j][rows], np.float16)
        in_maps.append(m)
    return in_maps, t, s, wq


def combine_partials(results, t, s, ce_weights):
    w = np.asarray(ce_weights, np.float64)
    A = 0.0
    Sc = np.zeros(C, np.float64)
    pen = 0.0
    for r in results:
        sums = r["sums"].astype(np.float64)
        A += sums[8].sum()
        Sc += sums[0:8].sum(axis=1)
        pen += sums[9].sum()
        accs = r["accs"].astype(np.float64)
        pfa, ha = accs[0:RB, 0], accs[0:RB, 1]
        pfb, hb = accs[RB:P, 0], accs[RB:P, 1]
        ua = ha - pfa
        ub = np.maximum(hb - pfb - ha, 0.0)
        pen += ((pfa + pfb) + 2.0 * (ua + ub)).sum()

    # clamped-tail correction for pair3/pair4 (reference clamps dot offsets
    # at S-2; the device codes clamp uniformly at S-1)
    i = np.arange(S - 4, S)
    d1r = s[:, np.minimum(i + 1, S - 2)]
    d2r = s[:, np.minimum(i + 2, S - 2)]
    r1 = s[:, np.minimum(i + 1, S - 1)]
    r2 = s[:, np.minimum(i + 2, S - 1)]
    r3 = s[:, np.minimum(i + 3, S - 1)]
    lp = s[:, i] == 1
    ref_p3 = lp & (d1r == 3) & (r2 == 2)
    ref_p4 = lp & (d1r == 3) & (d2r == 3) & (r3 == 2)
    dev_p3 = lp & (r1 == 3) & (r2 == 2)
    dev_p4 = lp & (r1 == 3) & (r2 == 3) & (r3 == 2)
    pen += (3.0 * (ref_p3.astype(np.float64) - dev_p3)
            + 4.0 * (ref_p4.astype(np.float64) - dev_p4)).sum()

    nnz = float((t != 0).sum())
    ce = (A - (w * Sc).sum()) / (B * S)
    penalty = pen / nnz
    return np.float32(ce + PENALTY_WEIGHT * penalty)


def kernel(logits, targets, predicted_structures, ce_weights):
    in_maps, t, s, wq = make_in_maps(
        logits, targets, predicted_structures, ce_weights)
    nc = _get_program()
    res = run_bass_kernel_spmd(nc, in_maps, core_ids=list(range(NCORES)))
    return combine_partials(res.results, t, s, ce_weights)
